# revision 31
# baseline (speedup 1.0000x reference)
"""Trainium2 Bass kernel for nn_MemoryEfficientS6Compressor (v5).

Math insight: the module output only depends on the last 8 sequence
positions of the LAST chunk, so we need:
  - xi (W_in proj) for chunk-local positions 14..31  (18 pos, 1152 tok)
  - conv+silu (xc) for positions 17..31              (15 pos,  960 tok)
  - dt / gate / window-softmax for positions 24..31  ( 8 pos,  512 tok)

Sharding: 7 conv groups -> cores 0..6 (core 7 zero weights). Per-core
channels padded 351->384 so every matmul M/K chunk is a full 128.
One AllReduce (xp partials, [32,512] fp32); final out-proj partials
summed + layernormed on the host during unshard.

v5 performance changes vs v4 (137us):
  - all weights/x pre-swizzled on host into [128, N] row-major DRAM
    tensors -> each streams as ONE big contiguous DMA (341+ GB/s
    instead of ~143 GB/s from 35 small row-chunk DMAs). x quarters on
    the sync queue, weights on the scalar queue, so A1 tracks the x
    stream and the xp AllReduce triggers at ~25us instead of ~63us.
  - phase D (window softmax) rewritten: merged [128, 3, 512] tiles
    (one DVE op covers all 3 channel chunks), Horner-pair window
    evaluation (xc0+r*xc1) + r2*(..) + r4*(..), affine_then_add fusing
    the D*xc_t term, ACT-engine Reciprocal for 1/S. No GpSimd
    tensor_tensor at all: GpSimd TT is 3.2x slower AND degrades
    concurrent Vector TT 3.3x (measured).
  - xps return copy on the sync HWDGE queue (0.6us vs 1us SWDGE).
  - keepalive matmuls threaded through phase D so the HAM clock gate
    keeps the PE at 2.4GHz for the final out-projection.
"""

import os

import numpy as np
import ml_dtypes

import concourse.bass as bass
import concourse.mybir as mybir
from concourse import bacc
from concourse.bass_utils import run_bass_kernel_spmd
from concourse.tile import TileContext

F32 = mybir.dt.float32
BF16 = mybir.dt.bfloat16
AF = mybir.ActivationFunctionType
ALU = mybir.AluOpType

SEQ, BATCH, D_MODEL = 128, 64, 2048
D_INNER, GROUPS, D_CONV = 2457, 7, 4
DT_RANK, WIN = 32, 8
GC = D_INNER // GROUPS           # 351 real channels per group
GCP = 384                        # padded to 3 full 128-chunks
NPOS = 18                        # xi positions (chunk-local 14..31)
NCONV = 15                       # conv output positions (17..31)
TOK = NPOS * BATCH               # 1152
TOKZ = WIN * BATCH               # 512
NK = D_MODEL // 128              # 16 k-chunks over d_model
NQ = 4                           # x DMA quarters (4 k-chunks each)

_cache = {}


def _build(stage="F"):
    nc = bacc.Bacc("TRN2", target_bir_lowering=False, debug=False,
                   num_devices=8)

    xts = [nc.dram_tensor(f"xt{q}", [128, 4 * TOK], BF16,
                          kind="ExternalInput").ap() for q in range(NQ)]
    win = nc.dram_tensor("win", [128, NK * GCP], BF16, kind="ExternalInput").ap()
    wgt = nc.dram_tensor("wgt", [128, NK * GCP], BF16, kind="ExternalInput").ap()
    wc = nc.dram_tensor("wc", [128, 3 * D_CONV * GCP], BF16,
                        kind="ExternalInput").ap()
    wdt = nc.dram_tensor("wdt", [DT_RANK, GCP], BF16, kind="ExternalInput").ap()
    wo = nc.dram_tensor("wo", [128, 3 * D_MODEL], BF16, kind="ExternalInput").ap()
    wx = nc.dram_tensor("wx", [128, 3 * DT_RANK], BF16, kind="ExternalInput").ap()
    biasv = nc.dram_tensor("biasv", [128, 9], F32, kind="ExternalInput").ap()
    dvec = nc.dram_tensor("dvec", [128, 3], F32, kind="ExternalInput").ap()
    bxp = nc.dram_tensor("bxp", [DT_RANK, 1], F32, kind="ExternalInput").ap()
    out = nc.dram_tensor("out", [BATCH, D_MODEL], F32, kind="ExternalOutput").ap()

    with TileContext(nc) as tc:
        with (
            tc.tile_pool(name="xt", bufs=1) as xt_pool,
            tc.tile_pool(name="wk", bufs=1) as wk_pool,
            tc.tile_pool(name="act", bufs=1) as act_pool,
            tc.tile_pool(name="ek", bufs=27) as ek_pool,
            tc.tile_pool(name="ekf", bufs=2) as ekf_pool,
            tc.tile_pool(name="sc", bufs=1) as sc_pool,
            tc.tile_pool(name="ps", bufs=1, space="PSUM") as ps_pool,
            tc.tile_pool(name="dram", bufs=1, space="DRAM") as dram_pool,
        ):
            # dummy tiny collective: absorbs the NEFF-entry barrier and the
            # first-collective stream startup so the real xp AllReduce
            # launches with ~1us trigger latency
            d_in = dram_pool.tile([DT_RANK, 1], F32, name="d_in")
            d_out = dram_pool.tile([DT_RANK, 1], F32, name="d_out")
            with tc.high_priority():
                nc.sync.dma_start(out=d_in[:], in_=bxp[:, :])
                nc.gpsimd.collective_compute(
                    "AllReduce", ALU.add,
                    replica_groups=[list(range(8))],
                    ins=[d_in.opt()], outs=[d_out.opt()])

            # ---- DMAs: the xp-critical stream (xt + win + wc, 7.5MB) is
            # balanced across both HWDGE queues; wgt/wo held back by a guard
            xt_sb = [xt_pool.tile([128, 4 * TOK], BF16, tag=f"xt{q}",
                                  name=f"xt{q}") for q in range(NQ)]
            for q in range(3):
                nc.sync.dma_start(out=xt_sb[q][:], in_=xts[q][:, :])
            bxp_sb = sc_pool.tile([DT_RANK, 1], F32, tag="bxp", name="bxp")
            nc.scalar.dma_start(out=bxp_sb[:], in_=bxp[:, :])
            win_sb = wk_pool.tile([128, NK * GCP], BF16, tag="win", name="win")
            for h in range(4):
                nc.scalar.dma_start(out=win_sb[:, 4 * h * GCP:4 * (h + 1) * GCP],
                                    in_=win[:, 4 * h * GCP:4 * (h + 1) * GCP])
            wc_sb = wk_pool.tile([128, 3 * D_CONV * GCP], BF16, tag="wc",
                                 name="wc")
            nc.scalar.dma_start(out=wc_sb[:], in_=wc[:, :])
            nc.scalar.dma_start(out=xt_sb[3][:], in_=xts[3][:, :])
            wx_sb = sc_pool.tile([128, 3 * DT_RANK], BF16, tag="wx", name="wx")
            nc.scalar.dma_start(out=wx_sb[:], in_=wx[:, :])
            biasv_sb = sc_pool.tile([128, 9], F32, tag="biasv", name="biasv")
            nc.scalar.dma_start(out=biasv_sb[:], in_=biasv[:, :])
            dvec_sb = sc_pool.tile([128, 3], F32, tag="dvec", name="dvec")
            nc.scalar.dma_start(out=dvec_sb[:], in_=dvec[:, :])
            wdt_sb = sc_pool.tile([DT_RANK, GCP], BF16, tag="wdt", name="wdt")
            nc.scalar.dma_start(out=wdt_sb[:], in_=wdt[:, :])
            # guard: hold the wgt/wo streams (not needed until ~t+45us) until
            # the xp-critical stream has fully landed, so they don't steal
            # HBM bandwidth from the AllReduce trigger path
            guard = sc_pool.tile([1, 1], BF16, tag="guard", name="guard")
            nc.scalar.activation(guard[:], xt_sb[2][0:1, 4 * TOK - 1:4 * TOK],
                                 AF.Copy)
            wgt_sb = wk_pool.tile([128, NK * GCP], BF16, tag="wgt", name="wgt")
            nc.scalar.dma_start(out=wgt_sb[:], in_=wgt[:, :])
            wo_sb = wk_pool.tile([128, 3 * D_MODEL], BF16, tag="wo", name="wo")
            nc.scalar.dma_start(out=wo_sb[:], in_=wo[:, :])

            def bias(i):
                return biasv_sb[:, i:i + 1]

            # ---- A1: xi tokens 448..1152 (positions 21..31) -----------------
            xi_sb = [act_pool.tile([128, TOK], BF16, tag=f"xi{m}",
                                   name=f"xi{m}") for m in range(3)]
            pa = [ps_pool.tile([128, 352], F32, tag=f"a{m}", bufs=1,
                               name=f"pa{m}") for m in range(3)]
            pb = [ps_pool.tile([128, 352], F32, tag=f"b{m}", bufs=1,
                               name=f"pb{m}") for m in range(3)]
            for q in range(NQ):
                for lk in range(4):
                    k = q * 4 + lk
                    st, sp_ = (k == 0), (k == NK - 1)
                    for m in range(3):
                        lhs = win_sb[:, k * GCP + m * 128:k * GCP + (m + 1) * 128]
                        nc.tensor.matmul(pa[m][:], lhs,
                                         xt_sb[q][:, lk * TOK + 448:lk * TOK + 800],
                                         start=st, stop=sp_)
                        nc.tensor.matmul(pb[m][:], lhs,
                                         xt_sb[q][:, lk * TOK + 800:lk * TOK + 1152],
                                         start=st, stop=sp_)
            for m in range(3):
                nc.scalar.activation(xi_sb[m][:, 448:800], pa[m][:],
                                     AF.Identity, bias=bias(m * 3 + 0))
                nc.scalar.activation(xi_sb[m][:, 800:1152], pb[m][:],
                                     AF.Identity, bias=bias(m * 3 + 0))

            if stage == "A":
                nc.sync.dma_start(out=out[0:64, 448:1152],
                                  in_=xi_sb[0][0:64, 448:1152])
                return nc

            # ---- convX + xp + AllReduce: highest scheduling priority --------
            # xcf cols 0..959 = conv positions 17..31 per m-chunk
            xcf = act_pool.tile([128, 3, 960], BF16, tag="xcf", name="xcf")
            cxtags = ["pc", "b0", "b1"]
            with tc.high_priority():
                for m in range(3):
                    pc = ps_pool.tile([128, 512], F32, tag=cxtags[m], bufs=1,
                                      name="pconv")
                    for kc in range(3):
                        for j in range(D_CONV):
                            nc.tensor.matmul(
                                pc[:],
                                wc_sb[:, kc * 1536 + j * GCP + m * 128:
                                      kc * 1536 + j * GCP + (m + 1) * 128],
                                xi_sb[kc][:, 448 + j * BATCH:960 + j * BATCH],
                                start=(kc == 0 and j == 0),
                                stop=(kc == 2 and j == D_CONV - 1))
                    nc.scalar.activation(xcf[:, m:m + 1, 448:960], pc[:],
                                         AF.Silu, bias=bias(m * 3 + 1))
                pxp = ps_pool.tile([DT_RANK, TOKZ], F32, tag="px", bufs=1,
                                   name="pxp")
                for kc in range(3):
                    nc.tensor.matmul(pxp[:],
                                     wx_sb[:, kc * DT_RANK:(kc + 1) * DT_RANK],
                                     xcf[:, kc:kc + 1, 448:960],
                                     start=(kc == 0), stop=(kc == 2))
                # bf16 payload: halves the ring bytes, and the return DMA
                # needs no cast so it rides the fast sync HWDGE queue
                xp_sb = sc_pool.tile([DT_RANK, TOKZ], BF16, tag="xp", name="xp")
                nc.scalar.activation(xp_sb[:], pxp[:], AF.Identity,
                                     bias=bxp_sb[:, 0:1])
                xp_part = dram_pool.tile([DT_RANK, TOKZ], BF16, name="xp_part")
                xp_red = dram_pool.tile([DT_RANK, TOKZ], BF16, name="xp_red")
                nc.sync.dma_start(out=xp_part[:], in_=xp_sb[:])
                nc.gpsimd.collective_compute(
                    "AllReduce", ALU.add,
                    replica_groups=[list(range(8))],
                    ins=[xp_part.opt()], outs=[xp_red.opt()])
                xps = sc_pool.tile([DT_RANK, TOKZ], BF16, tag="xps", name="xps")
                nc.sync.dma_start(out=xps[:], in_=xp_red[:])

            # ---- A2: xi tokens 0..448 (positions 14..20) --------------------
            for m in range(3):
                pa2 = ps_pool.tile([128, 448], F32, tag=f"a{m}", bufs=1,
                                   name="pa2")
                for q in range(NQ):
                    for lk in range(4):
                        k = q * 4 + lk
                        nc.tensor.matmul(
                            pa2[:],
                            win_sb[:, k * GCP + m * 128:k * GCP + (m + 1) * 128],
                            xt_sb[q][:, lk * TOK:lk * TOK + 448],
                            start=(k == 0), stop=(k == NK - 1))
                nc.scalar.activation(xi_sb[m][:, 0:448], pa2[:],
                                     AF.Identity, bias=bias(m * 3 + 0))

            # ---- conv2: conv tokens 0..448 (positions 17..23) ---------------
            c2tags = ["b0", "b1", "b2"]
            for m in range(3):
                pc2 = ps_pool.tile([128, 448], F32, tag=c2tags[m], bufs=1,
                                   name="pconv2")
                for kc in range(3):
                    for j in range(D_CONV):
                        nc.tensor.matmul(
                            pc2[:],
                            wc_sb[:, kc * 1536 + j * GCP + m * 128:
                                  kc * 1536 + j * GCP + (m + 1) * 128],
                            xi_sb[kc][:, j * BATCH:448 + j * BATCH],
                            start=(kc == 0 and j == 0),
                            stop=(kc == 2 and j == D_CONV - 1))
                nc.scalar.activation(xcf[:, m:m + 1, 0:448], pc2[:],
                                     AF.Silu, bias=bias(m * 3 + 1))

            def ekt(nm, dt=BF16):
                if dt is F32:
                    return ekf_pool.tile([128, 3, TOKZ], dt, tag="ekf",
                                         name=nm)
                return ek_pool.tile([128, 3, TOKZ], dt, tag="ek", name=nm)

            def xcs(k):
                return xcf[:, :, k * BATCH:k * BATCH + TOKZ]

            # ---- collective-independent phase D prep (during the wait) ------
            # window pair-sums s_i = xc(2i) + xc(2i+1); with r = 1+a the
            # Horner pair is P_i = xc(2i) + r*xc(2i+1) = s_i + a*xc(2i+1),
            # so the post-collective chain needs only a = e^pdt, not r.
            sA = []
            for i in range(4):
                s = ekt(f"s{i}")
                nc.vector.tensor_add(s[:], xcs(2 * i), xcs(2 * i + 1))
                sA.append(s)
            # D*xc_t term
            dz = ek_pool.tile([128, 3, TOKZ], BF16, tag="ek", name="dz")
            for m in range(3):
                nc.scalar.activation(dz[:, m:m + 1, :],
                                     xcf[:, m:m + 1, 448:960],
                                     AF.Identity, scale=dvec_sb[:, m:m + 1])
            # pre-warm the exp/square act table while the collective flies
            dumm = sc_pool.tile([DT_RANK, 1], F32, tag="dumm", name="dumm")
            nc.scalar.activation(dumm[:], bxp_sb[:], AF.Exp)

            # ---- gate m0, then pdt + EXPs (the dt chain starts right after
            # xps arrival while gate m1/m2 still run on the PE), then m1/m2
            sigz = act_pool.tile([128, 3, TOKZ], BF16, tag="sigz", name="sigz")

            def gate_m(m):
                pz = ps_pool.tile([128, TOKZ], F32, tag=f"a{m}", bufs=1,
                                  name=f"pz{m}")
                for q in range(NQ):
                    for lk in range(4):
                        k = q * 4 + lk
                        nc.tensor.matmul(
                            pz[:],
                            wgt_sb[:, k * GCP + m * 128:k * GCP + (m + 1) * 128],
                            xt_sb[q][:, lk * TOK + TOK - TOKZ:lk * TOK + TOK],
                            start=(k == 0), stop=(k == NK - 1))
                nc.scalar.activation(sigz[:, m:m + 1, :], pz[:],
                                     AF.Sigmoid, bias=bias(m * 3 + 2))

            gate_m(0)
            pdt = [ps_pool.tile([128, TOKZ], F32, tag=c2tags[m], bufs=1,
                                name=f"pdt{m}") for m in range(3)]
            for m in range(3):
                nc.tensor.matmul(pdt[m][:],
                                 wdt_sb[:, m * 128:(m + 1) * 128],
                                 xps[:], start=True, stop=True)
            a3 = ekt("a3")
            for m in range(3):
                nc.scalar.activation(a3[:, m:m + 1, :], pdt[m][:], AF.Exp)
            gate_m(1)
            r2 = ekt("r2")
            r4 = ekt("r4")
            r1 = ekt("r1")
            for m in range(3):
                nc.scalar.activation(r2[:, m:m + 1, :], a3[:, m:m + 1, :],
                                     AF.Square, bias=1.0)
            for m in range(3):
                nc.scalar.activation(r4[:, m:m + 1, :], r2[:, m:m + 1, :],
                                     AF.Square)
            nc.scalar.activation(r1[:], a3[:], AF.Identity, bias=1.0)
            gate_m(2)

            if stage == "B":
                nc.sync.dma_start(out=out[0:64, 0:960], in_=xcf[0:64, 0, :])
                return nc
            if stage == "C":
                nc.gpsimd.dma_start(out=out[0:32, 0:TOKZ], in_=xps[:])
                return nc

            # ---- phase D: windowed softmax attention ------------------------
            # w_k = r^k (k=0 oldest .. 7 newest), r = 1 + e^pdt;
            # S = (1+r)(1+r^2)(1+r^4); num via Horner pairs:
            #   num = (P0 + r2*P1) + r4*(P2 + r2*P3)
            # All elementwise on Vector (bf16 2x) + Scalar ACT only.
            ka_n = [0]

            def kalive(dep):
                ka = ps_pool.tile([DT_RANK, TOKZ], F32, tag="px", bufs=1,
                                  name=f"kalive{ka_n[0]}")
                ka_n[0] += 1
                nc.tensor.matmul(ka[:], wx_sb[:, 0:DT_RANK],
                                 dep[:, 0:1, :], start=True, stop=True)

            # window products start as soon as a = e^pdt is complete
            w1 = ekt("w1")
            nc.vector.tensor_mul(w1[:], a3[:], xcs(1))
            w3 = ekt("w3")
            nc.vector.tensor_mul(w3[:], a3[:], xcs(3))
            w5 = ekt("w5")
            nc.vector.tensor_mul(w5[:], a3[:], xcs(5))
            w7 = ekt("w7")
            nc.vector.tensor_mul(w7[:], a3[:], xcs(7))
            # denominator S = (1+r)(1+r^2)(1+r^4): bias-adds on Scalar
            b1 = ekt("b1")
            nc.scalar.activation(b1[:], r1[:], AF.Identity, bias=1.0)
            b2 = ekt("b2")
            nc.scalar.activation(b2[:], r2[:], AF.Identity, bias=1.0)
            b4 = ekt("b4")
            nc.scalar.activation(b4[:], r4[:], AF.Identity, bias=1.0)
            p0 = ekt("p0")
            nc.vector.tensor_add(p0[:], w1[:], sA[0][:])
            p1 = ekt("p1")
            nc.vector.tensor_add(p1[:], w3[:], sA[1][:])
            p2 = ekt("p2")
            nc.vector.tensor_add(p2[:], w5[:], sA[2][:])
            kalive(p1)
            p3 = ekt("p3")
            nc.vector.tensor_add(p3[:], w7[:], sA[3][:])
            sp = ekt("sp")
            nc.vector.tensor_mul(sp[:], b1[:], b2[:])
            q1 = ekt("q1")
            nc.vector.tensor_mul(q1[:], r2[:], p1[:])
            q3 = ekt("q3")
            nc.vector.tensor_mul(q3[:], r2[:], p3[:])
            kalive(q1)
            sv = ekt("sv")
            nc.vector.tensor_mul(sv[:], sp[:], b4[:])
            svf = ekt("svf", F32)
            nc.scalar.activation(svf[:], sv[:], AF.Copy)
            qq0 = ekt("qq0")
            nc.vector.tensor_add(qq0[:], p0[:], q1[:])
            qq1 = ekt("qq1")
            nc.vector.tensor_add(qq1[:], p2[:], q3[:])
            sinvf = ekt("sinvf", F32)
            nc.vector.reciprocal_approx_fast(out=sinvf[:], in_=svf[:])
            sinvb = ekt("sinvb")
            nc.scalar.activation(sinvb[:], sinvf[:], AF.Copy)
            kalive(qq1)
            mn = ekt("mn")
            nc.vector.tensor_mul(mn[:], r4[:], qq1[:])
            num = ekt("num")
            nc.vector.tensor_add(num[:], qq0[:], mn[:])
            qn = ekt("qn")
            nc.vector.tensor_mul(qn[:], num[:], sinvb[:])
            # dense burst: ~3.5us of sustained PE activity re-arms the HAM
            # clock gate so the phase E matmuls run at 2.4GHz, not 1.2
            for _ in range(3):
                kalive(num)
            ys0 = ekt("ys0")
            nc.vector.tensor_add(ys0[:], qn[:], dz[:])
            for _ in range(3):
                kalive(qn)
            ys = ekt("ys")
            nc.vector.tensor_mul(ys[:], ys0[:], sigz[:])
            for _ in range(2):
                kalive(ys0)
            t1 = ek_pool.tile([128, 3, 256], BF16, tag="ek", name="t1")
            nc.vector.tensor_add(t1[:], ys[:, :, 0:256], ys[:, :, 256:512])
            t2 = ek_pool.tile([128, 3, 128], BF16, tag="ek", name="t2")
            nc.vector.tensor_add(t2[:], t1[:, :, 0:128], t1[:, :, 128:256])
            t3 = ek_pool.tile([128, 3, 64], BF16, tag="ek", name="t3")
            nc.vector.tensor_add(t3[:], t2[:, :, 0:64], t2[:, :, 64:128])

            if stage == "D":
                for m in range(3):
                    nc.sync.dma_start(out=out[0:128, m * 64:(m + 1) * 64],
                                      in_=t3[:, m, :])
                return nc

            # ---- phase E: out partial = cext @ woT --------------------------
            po_tags = ["a0", "a1", "a2", "pc"]
            po = [ps_pool.tile([BATCH, 512], F32, tag=po_tags[n], bufs=1,
                               name=f"po{n}") for n in range(4)]
            outp = sc_pool.tile([BATCH, D_MODEL], F32, tag="outp", name="outp")
            for n in range(4):
                for kc in range(3):
                    nc.tensor.matmul(po[n][:], t3[:, kc:kc + 1, :],
                                     wo_sb[:, kc * D_MODEL + n * 512:
                                           kc * D_MODEL + (n + 1) * 512],
                                     start=(kc == 0), stop=(kc == 2))
                nc.scalar.activation(outp[:, n * 512:(n + 1) * 512],
                                     po[n][:], AF.Copy)
                nc.sync.dma_start(out=out[:, n * 512:(n + 1) * 512],
                                  in_=outp[:, n * 512:(n + 1) * 512])

    nc.compile()
    return nc


def _host_prep(inputs):
    f = lambda k: np.ascontiguousarray(np.asarray(inputs[k], dtype=np.float32))
    x, W_in, b_in = f("x"), f("W_in"), f("b_in")
    W_gate, b_gate = f("W_gate"), f("b_gate")
    W_conv, b_conv = f("W_conv"), f("b_conv")
    W_xproj, b_xproj = f("W_xproj"), f("b_xproj")
    W_dt, Dparam = f("W_dt"), f("Dparam")
    W_out = f("W_out")

    bf = lambda a: np.ascontiguousarray(a.astype(ml_dtypes.bfloat16))

    def swz(a):
        # [2048, C] -> [128, 16*C] with k-chunk-major columns
        c = a.shape[1]
        return np.ascontiguousarray(
            a.reshape(NK, 128, c).transpose(1, 0, 2).reshape(128, NK * c))

    def swzc(a):
        # [384, C] (channel rows) -> [128, 3*C]
        c = a.shape[1]
        return np.ascontiguousarray(
            a.reshape(3, 128, c).transpose(1, 0, 2).reshape(128, 3 * c))

    # stacked identity: 4 copies of I_32 -> [128, 32] (AllGather local sum)
    idm = np.zeros((128, DT_RANK), np.float32)
    idm[np.arange(128), np.arange(128) % DT_RANK] = 1.0

    # x^T swizzled into 4 quarters of 4 k-chunks each
    xT = x[SEQ - NPOS:].reshape(TOK, D_MODEL).T            # [2048, 1152]
    xT = bf(xT).reshape(NK, 128, TOK)
    xqs = [np.ascontiguousarray(
        xT[q * 4:(q + 1) * 4].transpose(1, 0, 2).reshape(128, 4 * TOK))
        for q in range(NQ)]

    in_maps = []
    for g in range(8):
        if g < GROUPS:
            ch = slice(GC * g, GC * (g + 1))
            wip = np.zeros((GCP, D_MODEL), np.float32)
            wip[:GC] = W_in[ch]
            wgp = np.zeros((GCP, D_MODEL), np.float32)
            wgp[:GC] = W_gate[ch]
            wcp = np.zeros((GCP, GCP, D_CONV), np.float32)
            wcp[:GC, :GC] = W_conv[ch]
            wdp = np.zeros((GCP, DT_RANK), np.float32)
            wdp[:GC] = W_dt[ch]
            wxp = np.zeros((GCP, DT_RANK), np.float32)
            wxp[:GC] = W_xproj[:DT_RANK, ch].T
            wop = np.zeros((GCP, D_MODEL), np.float32)
            wop[:GC] = W_out[:, ch].T / float(WIN)
            bip = np.zeros((GCP, 3), np.float32)
            bip[:GC, 0] = b_in[ch]
            bip[:GC, 1] = b_conv[ch]
            bip[:GC, 2] = b_gate[ch]
            dvp = np.zeros((GCP,), np.float32)
            dvp[:GC] = Dparam[ch]

            winm = swz(bf(wip.T))                          # [128, 16*384]
            wgtm = swz(bf(wgp.T))
            # wc: [in(3,128), tap, out] -> [128, 3*1536]
            wcm = swzc(bf(wcp.transpose(1, 2, 0).reshape(GCP, D_CONV * GCP)))
            wdtm = bf(wdp.T)                               # [32, 384]
            wxm = swzc(bf(wxp))                            # [128, 3*32]
            wom = swzc(bf(wop))                            # [128, 3*2048]
            biasm = np.ascontiguousarray(
                bip.reshape(3, 128, 3).transpose(1, 0, 2).reshape(128, 9))
            dvm = np.ascontiguousarray(dvp.reshape(3, 128).T)
            bxpm = (b_xproj[:DT_RANK] if g == 0
                    else np.zeros(DT_RANK, np.float32)).reshape(DT_RANK, 1)
            bxpm = np.ascontiguousarray(bxpm)
        else:
            winm = np.zeros((128, NK * GCP), ml_dtypes.bfloat16)
            wgtm = np.zeros((128, NK * GCP), ml_dtypes.bfloat16)
            wcm = np.zeros((128, 3 * D_CONV * GCP), ml_dtypes.bfloat16)
            wdtm = np.zeros((DT_RANK, GCP), ml_dtypes.bfloat16)
            wom = np.zeros((128, 3 * D_MODEL), ml_dtypes.bfloat16)
            wxm = np.zeros((128, 3 * DT_RANK), ml_dtypes.bfloat16)
            biasm = np.zeros((128, 9), np.float32)
            dvm = np.zeros((128, 3), np.float32)
            bxpm = np.zeros((DT_RANK, 1), np.float32)
        im = {"win": winm, "wgt": wgtm, "wc": wcm, "wdt": wdtm,
              "wo": wom, "wx": wxm, "biasv": biasm, "dvec": dvm,
              "bxp": bxpm, "ident": idm}
        for q in range(NQ):
            im[f"xt{q}"] = xqs[q]
        in_maps.append(im)
    return in_maps


def _finish(res, inputs):
    """gather/unshard: sum the per-group out partials, add b_out, layernorm"""
    acc = np.zeros((BATCH, D_MODEL), np.float64)
    for g in range(GROUPS):
        acc += res.results[g]["out"].astype(np.float64)
    o = acc.astype(np.float32) + np.asarray(inputs["b_out"], np.float32)
    mu = o.mean(-1, keepdims=True)
    var = ((o - mu) ** 2).mean(-1, keepdims=True)
    o = (o - mu) / np.sqrt(var + 1e-5)
    o = o * np.asarray(inputs["ln_w"], np.float32) + np.asarray(
        inputs["ln_b"], np.float32)
    return o.astype(np.float32)


def kernel(**inputs):
    if "nc" not in _cache:
        _cache["nc"] = _build(os.environ.get("K_STAGE", "F"))
    in_maps = _host_prep(inputs)
    res = run_bass_kernel_spmd(_cache["nc"], in_maps, list(range(8)))
    if os.environ.get("K_STAGE", "F") != "F":
        return res.results[0]["out"]
    return _finish(res, inputs)
